# revision 17
# baseline (speedup 1.0000x reference)
"""Trainium2 Bass kernel: 2-layer LSTM (T=80, H=256) + embedding + softmax CE loss.

Strategy: data-parallel over batch (8192 -> 8 cores x 1024).  Everything runs
in a transposed layout: states/gates keep the hidden/gate dim on SBUF
partitions and the batch dim on the free axis, so the recurrent matmuls need
no per-step transposes (stationary = weights, moving = state).

The recurrent h-matmuls run in fp8e4m3 with perf_mode=DoubleRow (2
contraction rows packed per partition -> half the PE instructions and 2x ALU
rate).  Weights are pre-scaled by 8 on the host (escapes the e4m3 subnormal
floor); the gate activation applies scale=1/8 plus the fp32 bias (b1/b2,
forget+1).

The embedding lookup is a one-hot matmul kept in bf16 (exact one-hot, cheap
FWL weight loads): x_t @ W1x == onehot(feat_t) @ E1 with E1 = 8*(emb @ W1x)
precomputed on the host.

A logical [256, 1024] tensor is stored "folded" as one SBUF tile [128, 2048]:
hidden unit u lives at (partition u % 128, col-block u // 128).  The fold
blocks double as the two DoubleRow k-tiles.  States h1/h2 are stored fp8
(they only feed matmuls); c1/c2 and gate activations stay bf16.

Emission order per step: L1 matmuls/acts, L1 cell (so tanh(c1) does not queue
behind L2's gate acts on the ACT engine), onehot prefetch for t+1, then L2 of
step t-1 (software pipeline: L2 runs one step behind L1).

Final loss: the last h2 is recomputed in bf16, logits = Wd.T @ h2 -> [80, B],
PE-transposed back to [B-chunk, 80] so log-sum-exp and the label gather run
along the free axis.
"""

import sys

sys.path.insert(0, "/opt/trn_rl_repo")

import numpy as np

import concourse.bass as bass
import concourse.mybir as mybir
import concourse.tile as tile
from concourse import bacc
from concourse.bass_utils import run_bass_kernel_spmd

AF = mybir.ActivationFunctionType
OP = mybir.AluOpType
F32 = mybir.dt.float32
BF16 = mybir.dt.bfloat16
F8 = mybir.dt.float8e4
DR = mybir.MatmulPerfMode.DoubleRow
DRS = mybir.MatmulPerfMode.DoubleRowSwInterleave
SWI = True         # software-interleaved weights: contiguous (FWL-speed) loads
DT = BF16          # dtype for gate activations / c state
WSCALE = 8.0       # host-side weight scale (fp8 subnormal avoidance)
INV = 1.0 / WSCALE

P = 128          # partitions
N_CORES = 8
B = 1024         # per-core batch shard
T = 80           # seq len
C = 80           # num classes
E = 8            # emb dim
H = 256          # hidden
G = 4 * H        # gates = 1024
NB = B // 512    # moving-operand chunks of 512 (psum bank limit)

GATE_FUNCS = [AF.Sigmoid, AF.Tanh, AF.Sigmoid, AF.Sigmoid]  # i, j, f, o


def build_program(T_steps=T, thin=None):
    # Bacc (not plain Bass): its compile() runs generate_event_semaphores,
    # which splits excess per-instruction sync waits onto InstEventSemaphore
    # nops — walrus only allows one wait on LDWEIGHTS/MATMULT.
    nc = bacc.Bacc("TRN2", target_bir_lowering=False, debug=False,
                   enable_asserts=False, num_devices=N_CORES)

    # ---------------- DRAM I/O ----------------
    # inputs are packed into 4 tensors — each extra input tensor costs
    # ~10us of per-call runtime overhead
    featrep = nc.dram_tensor("featrep", [T_steps, C, B], DT, kind="ExternalInput").ap()
    w8d = nc.dram_tensor("w8", [P, 6 * G], F8, kind="ExternalInput").ap()
    cf = nc.dram_tensor("cf32", [P, 186], F32, kind="ExternalInput").ap()
    cb = nc.dram_tensor("cbf", [2 * P, G + C], DT, kind="ExternalInput").ap()
    lossd = nc.dram_tensor("loss", [P, B // P], F32, kind="ExternalOutput").ap()

    W1hd, W2ad, W2bd = (w8d[:, 0:2 * G], w8d[:, 2 * G:4 * G], w8d[:, 4 * G:6 * G])
    b1cd, b2cd = cf[:, 0:8], cf[:, 8:16]
    iotalabd, labelsT = cf[:, 16:96], cf[:, 96:96 + B // P]
    ident, iota80d, bdd = cf[0:C, 104:184], cf[0:C, 184:185], cf[0:C, 185:186]
    E1d, Wdd = cb[0:C, 0:G], cb[:, G:G + C]

    with tile.TileContext(nc) as tc:
        _emit(nc, tc, featrep, labelsT, E1d, W1hd, W2ad, W2bd, b1cd, b2cd,
              Wdd, bdd, ident, iota80d, iotalabd, lossd, T_steps, thin)
    nc.compile()
    return nc


def _emit(nc, tc, featrep, labelsT, E1d, W1hd, W2ad, W2bd, b1cd, b2cd,
          Wdd, bdd, ident, iota80d, iotalabd, lossd, T_steps=T, thin=None):
    f32 = F32

    def act(out, in_, func, **kw):
        if thin == "act":
            nc.scalar.activation(out[:, 0:32], in_[:, 0:32], func, **kw)
        else:
            nc.scalar.activation(out, in_, func, **kw)

    def tt(out, a, b_, op):
        if thin == "dve":
            nc.vector.tensor_tensor(out[:, 0:32], a[:, 0:32], b_[:, 0:32], op=op)
        else:
            nc.vector.tensor_tensor(out, a, b_, op=op)

    def drm(t2d, n):
        # [p, 2B] state tile -> [p, 2, 512] DoubleRow moving view
        return t2d.rearrange("p (i b) -> p i b", i=2)[:, :, 512 * n:512 * (n + 1)]

    def drw(w2d, m):
        # [p, 2G] weight tile -> [p, 2, 128] DoubleRow stationary view
        if SWI:
            return w2d[:, 2 * P * m:2 * P * (m + 1)].rearrange(
                "p (i g) -> p i g", i=2)
        return w2d.rearrange("p (i g) -> p i g", i=2)[:, :, P * m:P * (m + 1)]

    DRM = DRS if SWI else DR

    const = tc.alloc_tile_pool(name="const", bufs=1)

    # ---------------- resident weights/constants ----------------
    E1 = const.tile([C, G], DT)
    nc.sync.dma_start(out=E1, in_=E1d)
    W1h = const.tile([P, 2 * G], F8)
    nc.sync.dma_start(out=W1h, in_=W1hd)
    W2a = const.tile([P, 2 * G], F8)
    nc.sync.dma_start(out=W2a, in_=W2ad)
    W2b = const.tile([P, 2 * G], F8)
    nc.sync.dma_start(out=W2b, in_=W2bd)
    b1c = const.tile([P, 8], f32)
    nc.sync.dma_start(out=b1c, in_=b1cd)
    b2c = const.tile([P, 8], f32)
    nc.sync.dma_start(out=b2c, in_=b2cd)
    Wd = const.tile([P, 2 * C], DT)
    for k in range(2):
        nc.sync.dma_start(out=Wd[:, k * C:(k + 1) * C],
                          in_=Wdd[P * k: P * (k + 1), :])
    bdc = const.tile([C, 1], f32)
    nc.sync.dma_start(out=bdc, in_=bdd)
    id80 = const.tile([C, C], f32)
    nc.sync.dma_start(out=id80, in_=ident)
    iotalab = const.tile([P, C], f32)
    nc.sync.dma_start(out=iotalab, in_=iotalabd)
    labT = const.tile([P, B // P], f32)
    nc.sync.dma_start(out=labT, in_=labelsT)

    # ---------------- pools for the recurrent loop ----------------
    states = tc.alloc_tile_pool(name="states", bufs=2)
    gates = tc.alloc_tile_pool(name="gates", bufs=2)
    pgate = tc.alloc_tile_pool(name="pgate", bufs=4, space="PSUM")
    feats = tc.alloc_tile_pool(name="feats", bufs=3)

    h1 = c1 = h2 = c2 = None

    def oh_build(t):
        # featrep already holds the one-hot (built host-side)
        oh = feats.tile([C, B], DT, tag="oh", name=f"oh_{t}")
        nc.sync.dma_start(out=oh, in_=featrep[t])
        return oh

    def l1_block(t, oh, h1_in):
        sg1 = []
        for g in range(4):
            sg = gates.tile([P, 2 * B], DT, tag=f"sg_{g}", name=f"sg1_{g}")
            for ml in range(2):
                m = 2 * g + ml
                ps = pgate.tile([P, B], f32, tag="g", name=f"ps1_{t}_{g}_{ml}")
                for n in range(NB):
                    nc.tensor.matmul(ps[:, 512 * n: 512 * (n + 1)],
                                     E1[:, P * m: P * (m + 1)],
                                     oh[:, 512 * n: 512 * (n + 1)],
                                     start=True, stop=(t == 0))
                if t > 0 and thin != "pe":
                    for n in range(NB):
                        nc.tensor.matmul(ps[:, 512 * n: 512 * (n + 1)],
                                         drw(W1h, m), drm(h1_in, n),
                                         start=False, stop=True, perf_mode=DRM)
                act(sg[:, ml * B:(ml + 1) * B], ps, GATE_FUNCS[g],
                    bias=b1c[:, m:m + 1], scale=INV)
            sg1.append(sg)
        return sg1

    def l2_block(t, h1_in, h2_in):
        sg2 = []
        for g in range(4):
            sg = gates.tile([P, 2 * B], DT, tag=f"sg_{g}", name=f"sg2_{g}")
            for ml in range(2):
                m = 2 * g + ml
                ps = pgate.tile([P, B], f32, tag="g", name=f"ps2_{t}_{g}_{ml}")
                for n in range(NB):
                    nc.tensor.matmul(ps[:, 512 * n: 512 * (n + 1)],
                                     drw(W2a, m), drm(h1_in, n),
                                     start=True, stop=(t == 0), perf_mode=DRM)
                if t > 0 and thin != "pe":
                    for n in range(NB):
                        nc.tensor.matmul(ps[:, 512 * n: 512 * (n + 1)],
                                         drw(W2b, m), drm(h2_in, n),
                                         start=False, stop=True, perf_mode=DRM)
                act(sg[:, ml * B:(ml + 1) * B], ps,
                    GATE_FUNCS[g], bias=b2c[:, m:m + 1], scale=INV)
            sg2.append(sg)
        return sg2

    def cell(t, sgates, c_in, ctag, htag, thname):
        # Chunked along batch so chunk 0's h is ready for the next step's
        # matmuls while chunk 1 is still flowing through DVE/ACT.
        si, sj, sf, so = sgates
        cn = states.tile([P, 2 * B], DT, tag=ctag, name=ctag)
        th = gates.tile([P, 2 * B], DT, tag="th", name=thname, bufs=2)
        hn = states.tile([P, 2 * B], F8, tag=htag, name=htag)
        for n in range(NB):
            if thin in ("dve", "act"):
                v = lambda t2d: t2d[:, 64 * n:64 * n + 32]
            else:
                v = lambda t2d: drm(t2d, n)
            nc.vector.tensor_tensor(v(si), v(si), v(sj), op=OP.mult)
            if t == 0:
                nc.vector.tensor_copy(v(cn), v(si))
            else:
                nc.vector.tensor_tensor(v(sf), v(c_in), v(sf), op=OP.mult)
                nc.vector.tensor_tensor(v(cn), v(sf), v(si), op=OP.add)
            nc.scalar.activation(v(th), v(cn), AF.Tanh)
            nc.vector.tensor_tensor(v(hn), v(th), v(so), op=OP.mult)
        return cn, hn, th, so

    # Software pipeline: L2 runs one step behind L1.  L1's cell is emitted
    # before L2's block so tanh(c1) is not queued behind L2's gate acts.
    oh = oh_build(0)
    sg2_pend = None
    for t in range(T_steps):
        h1_prev = h1
        sg1 = l1_block(t, oh, h1_prev)
        c1, h1, _, _ = cell(t, sg1, c1, "c1", "h1", "th1")
        if t + 1 < T_steps:
            oh = oh_build(t + 1)                 # prefetch, off the h1 chain
        if t > 0:
            sg2_pend = l2_block(t - 1, h1_prev, h2)
            c2, h2, _, _ = cell(t - 1, sg2_pend, c2, "c2", "h2", "th2")

    # drain the pipeline: L2 for the final step
    sg2_pend = l2_block(T_steps - 1, h1, h2)
    c2, h2, th2, so2 = cell(T_steps - 1, sg2_pend, c2, "c2", "h2", "th2")

    # recompute the final h2 in bf16 for the dense layer
    h2bf = gates.tile([P, 2 * B], DT, tag="h2bf", bufs=1)
    nc.vector.tensor_tensor(h2bf, th2, so2, op=OP.mult)

    feats.release()
    pgate.release()

    # ---------------- loss ----------------
    ploss = tc.alloc_tile_pool(name="ploss", bufs=1, space="PSUM")
    lpool = tc.alloc_tile_pool(name="lpool", bufs=2)

    lps = ploss.tile([C, B], f32, tag="logits")
    for n in range(NB):
        for k in range(2):
            nc.tensor.matmul(
                lps[:, 512 * n: 512 * (n + 1)],
                Wd[:, C * k: C * (k + 1)],
                h2bf[:, B * k + 512 * n: B * k + 512 * (n + 1)],
                start=(k == 0), stop=(k == 1))
    logits = lpool.tile([C, B], f32, tag="logits_sb", bufs=1)
    nc.scalar.activation(logits, lps, AF.Identity, bias=bdc[:, 0:1])

    loss_sb = lpool.tile([P, B // P], f32, tag="loss_sb", bufs=1)
    # per-chunk sum-exps / label logits gathered as COLUMNS of shared tiles so
    # the log and the final subtract are single ops (one Exp->Ln table switch)
    sumexp_all = lpool.tile([P, B // P], f32, tag="sumexp_all", bufs=1)
    lablog_all = lpool.tile([P, B // P], f32, tag="lablog_all", bufs=1)
    for cb in range(B // P):
        lt = ploss.tile([P, C], f32, tag="lt", bufs=2, name=f"lt_{cb}")
        nc.tensor.transpose(lt, logits[:, P * cb: P * (cb + 1)], id80)
        ohl = lpool.tile([P, C], f32, tag="ohl", name=f"ohl_{cb}")
        nc.vector.tensor_scalar(ohl, iotalab, labT[:, cb:cb + 1], None,
                                op0=OP.is_equal)
        scr1 = lpool.tile([P, C], f32, tag="scr1", name=f"scr1_{cb}")
        nc.vector.scalar_tensor_tensor(scr1, lt, 1.0, ohl,
                                       op0=OP.mult, op1=OP.mult,
                                       accum_out=lablog_all[:, cb:cb + 1])
        scr2 = lpool.tile([P, C], f32, tag="scr2", name=f"scr2_{cb}")
        nc.scalar.activation(scr2, lt, AF.Exp,
                             accum_out=sumexp_all[:, cb:cb + 1])
    lse = lpool.tile([P, B // P], f32, tag="lse", bufs=1)
    nc.scalar.activation(lse, sumexp_all, AF.Ln)
    nc.vector.tensor_sub(loss_sb, lse, lablog_all)
    nc.sync.dma_start(out=lossd, in_=loss_sb)
    lpool.release()
    ploss.release()
    gates.release()
    states.release()
    const.release()


# ---------------------------------------------------------------------------
# host side
# ---------------------------------------------------------------------------
_CACHE = {}


def _get_program():
    if "nc" not in _CACHE:
        _CACHE["nc"] = build_program()
    return _CACHE["nc"]


def make_in_maps(features, labels, embedding, W1, b1, W2, b2, Wd, bd):
    """Shard the full inputs into 8 per-core input maps."""
    import ml_dtypes
    f32 = np.float32
    fp8 = ml_dtypes.float8_e4m3
    bf16 = ml_dtypes.bfloat16
    features = np.asarray(features, dtype=np.int32)
    labels = np.asarray(labels, dtype=np.int32)
    emb32 = np.asarray(embedding, f32)
    W132 = np.asarray(W1, f32)
    W232 = np.asarray(W2, f32)

    def fold2(mat):  # [256, G] -> [128, 2G] with k-tile-major columns
        if SWI:
            # per gate-block m: [A127, B127, ..., A0, B0] (A/B = k-tiles,
            # columns reversed) — the hw DoubleRowSwInterleave weight layout
            wf = mat.reshape(2, P, 8, P)[..., ::-1]
            return np.ascontiguousarray(
                wf.transpose(1, 2, 3, 0).reshape(P, 2 * G))
        return np.ascontiguousarray(
            mat.reshape(2, P, G).transpose(1, 0, 2).reshape(P, 2 * G))

    E1bf = np.ascontiguousarray(WSCALE * (emb32 @ W132[:E])).astype(bf16)
    W1h8 = fold2(WSCALE * W132[E:]).astype(fp8)
    W28a = fold2(WSCALE * W232[:H]).astype(fp8)
    W28b = fold2(WSCALE * W232[H:]).astype(fp8)
    b1c = np.ascontiguousarray(np.asarray(b1, f32).reshape(8, P).T)
    b1c[:, 4:6] += 1.0                          # forget-gate bias
    b2c = np.ascontiguousarray(np.asarray(b2, f32).reshape(8, P).T)
    b2c[:, 4:6] += 1.0

    w8 = np.ascontiguousarray(np.concatenate([W1h8, W28a, W28b], axis=1))
    cbf = np.zeros((2 * P, G + C), bf16)
    cbf[0:C, 0:G] = E1bf
    cbf[:, G:G + C] = np.asarray(Wd, f32).astype(bf16)
    cf32 = np.zeros((P, 186), f32)
    cf32[:, 0:8] = b1c
    cf32[:, 8:16] = b2c
    cf32[:, 16:96] = np.arange(C, dtype=f32)[None, :]
    cf32[0:C, 104:184] = np.eye(C, dtype=f32)
    cf32[0:C, 184] = np.arange(C, dtype=f32)
    cf32[0:C, 185] = np.asarray(bd, f32)

    in_maps = []
    for c in range(N_CORES):
        fs = features[B * c: B * (c + 1)]            # [B, T]
        ls = labels[B * c: B * (c + 1)]              # [B]
        ft = fs.T                                    # [T, B]
        featrep = np.ascontiguousarray(
            (ft[:, None, :] == np.arange(C, dtype=np.int32)[None, :, None])
            .astype(bf16))
        cfc = cf32.copy()
        cfc[:, 96:96 + B // P] = ls.reshape(B // P, P).T.astype(f32)
        in_maps.append({"w8": w8, "cbf": cbf, "cf32": cfc, "featrep": featrep})
    return in_maps


def gather_output(results):
    outs = []
    for r in results:
        outs.append(np.asarray(r["loss"]).T.reshape(-1))   # [P, B//P] -> [B]
    return np.concatenate(outs, axis=0).astype(np.float32)


def kernel(features, labels, embedding, W1, b1, W2, b2, Wd, bd):
    nc = _get_program()
    in_maps = make_in_maps(features, labels, embedding, W1, b1, W2, b2, Wd, bd)
    res = run_bass_kernel_spmd(nc, in_maps, core_ids=list(range(N_CORES)))
    return gather_output(res.results)


# revision 23
# speedup vs baseline: 1.0115x; 1.0115x over previous
"""Trainium2 Bass kernel: 2-layer LSTM (T=80, H=256) + embedding + softmax CE loss.

Strategy: data-parallel over batch (8192 -> 8 cores x 1024).  Everything runs
in a transposed layout: states/gates keep the hidden/gate dim on SBUF
partitions and the batch dim on the free axis, so the recurrent matmuls need
no per-step transposes (stationary = weights, moving = state).

The recurrent h-matmuls run in fp8e4m3 with perf_mode=DoubleRow (2
contraction rows packed per partition -> half the PE instructions and 2x ALU
rate).  Weights are pre-scaled by 8 on the host (escapes the e4m3 subnormal
floor); the gate activation applies scale=1/8 plus the fp32 bias (b1/b2,
forget+1).

The embedding lookup is a one-hot matmul kept in bf16 (exact one-hot, cheap
FWL weight loads): x_t @ W1x == onehot(feat_t) @ E1 with E1 = 8*(emb @ W1x)
precomputed on the host.

A logical [256, 1024] tensor is stored "folded" as one SBUF tile [128, 2048]:
hidden unit u lives at (partition u % 128, col-block u // 128).  The fold
blocks double as the two DoubleRow k-tiles.  States h1/h2 are stored fp8
(they only feed matmuls); c1/c2 and gate activations stay bf16.

Emission order per step: L1 matmuls/acts, L1 cell (so tanh(c1) does not queue
behind L2's gate acts on the ACT engine), onehot prefetch for t+1, then L2 of
step t-1 (software pipeline: L2 runs one step behind L1).

Final loss: the last h2 is recomputed in bf16, logits = Wd.T @ h2 -> [80, B],
PE-transposed back to [B-chunk, 80] so log-sum-exp and the label gather run
along the free axis.
"""

import sys

sys.path.insert(0, "/opt/trn_rl_repo")

import numpy as np

import concourse.bass as bass
import concourse.mybir as mybir
import concourse.tile as tile
from concourse import bacc
from concourse.bass_utils import run_bass_kernel_spmd

AF = mybir.ActivationFunctionType
OP = mybir.AluOpType
F32 = mybir.dt.float32
BF16 = mybir.dt.bfloat16
F8 = mybir.dt.float8e4
DR = mybir.MatmulPerfMode.DoubleRow
DRS = mybir.MatmulPerfMode.DoubleRowSwInterleave
SWI = True         # software-interleaved weights: contiguous (FWL-speed) loads
DT = BF16          # dtype for gate activations / c state
WSCALE = 8.0       # host-side weight scale (fp8 subnormal avoidance)
INV = 1.0 / WSCALE

P = 128          # partitions
N_CORES = 8
B = 1024         # per-core batch shard
T = 80           # seq len
C = 80           # num classes
E = 8            # emb dim
H = 256          # hidden
G = 4 * H        # gates = 1024
NB = B // 512    # moving-operand chunks of 512 (psum bank limit)

GATE_FUNCS = [AF.Sigmoid, AF.Tanh, AF.Sigmoid, AF.Sigmoid]  # i, j, f, o


def build_program(T_steps=T, thin=None):
    # Bacc (not plain Bass): its compile() runs generate_event_semaphores,
    # which splits excess per-instruction sync waits onto InstEventSemaphore
    # nops — walrus only allows one wait on LDWEIGHTS/MATMULT.
    nc = bacc.Bacc("TRN2", target_bir_lowering=False, debug=False,
                   enable_asserts=False, num_devices=N_CORES)

    # ---------------- DRAM I/O ----------------
    # inputs are packed into 2 tensors — each extra input tensor costs
    # ~10us of per-call runtime overhead.  The weights/constants blob is
    # declared bf16 and per-region bitcast to fp8/f32 (byte-identical).
    featrep = nc.dram_tensor("featrep", [T_steps, C, B], DT, kind="ExternalInput").ap()
    blob = nc.dram_tensor("blob", [2 * P, 4548], DT, kind="ExternalInput").ap()
    lossd = nc.dram_tensor("loss", [P, B // P], F32, kind="ExternalOutput").ap()

    w8d = blob[0:P, 0:3 * G].bitcast(F8)            # [P, 6G] fp8
    cf = blob[0:P, 3 * G:3 * G + 372].bitcast(F32)  # [P, 186] f32
    cb = blob[:, 3 * G + 372:3 * G + 372 + G + C]   # [2P, G+C] bf16

    W1hd, W2ad, W2bd = (w8d[:, 0:2 * G], w8d[:, 2 * G:4 * G], w8d[:, 4 * G:6 * G])
    b1cd, b2cd = cf[:, 0:8], cf[:, 8:16]
    iotalabd, labelsT = cf[:, 16:96], cf[:, 96:96 + B // P]
    ident, iota80d, bdd = cf[0:C, 104:184], cf[0:C, 184:185], cf[0:C, 185:186]
    E1d, Wdd = cb[0:C, 0:G], cb[:, G:G + C]

    with tile.TileContext(nc) as tc:
        _emit(nc, tc, featrep, labelsT, E1d, W1hd, W2ad, W2bd, b1cd, b2cd,
              Wdd, bdd, ident, iota80d, iotalabd, lossd, T_steps, thin)
    nc.compile()
    return nc


def _emit(nc, tc, featrep, labelsT, E1d, W1hd, W2ad, W2bd, b1cd, b2cd,
          Wdd, bdd, ident, iota80d, iotalabd, lossd, T_steps=T, thin=None):
    f32 = F32

    def act(out, in_, func, **kw):
        if thin == "act":
            nc.scalar.activation(out[:, 0:32], in_[:, 0:32], func, **kw)
        else:
            nc.scalar.activation(out, in_, func, **kw)

    def tt(out, a, b_, op):
        if thin == "dve":
            nc.vector.tensor_tensor(out[:, 0:32], a[:, 0:32], b_[:, 0:32], op=op)
        else:
            nc.vector.tensor_tensor(out, a, b_, op=op)

    def drm(t2d, n):
        # [p, 2B] state tile -> [p, 2, 512] DoubleRow moving view
        return t2d.rearrange("p (i b) -> p i b", i=2)[:, :, 512 * n:512 * (n + 1)]

    def drw(w2d, m):
        # [p, 2G] weight tile -> [p, 2, 128] DoubleRow stationary view
        if SWI:
            return w2d[:, 2 * P * m:2 * P * (m + 1)].rearrange(
                "p (i g) -> p i g", i=2)
        return w2d.rearrange("p (i g) -> p i g", i=2)[:, :, P * m:P * (m + 1)]

    DRM = DRS if SWI else DR

    const = tc.alloc_tile_pool(name="const", bufs=1)

    # ---------------- resident weights/constants ----------------
    E1 = const.tile([C, G], DT)
    nc.sync.dma_start(out=E1, in_=E1d)
    W1h = const.tile([P, 2 * G], F8)
    nc.sync.dma_start(out=W1h, in_=W1hd)
    W2a = const.tile([P, 2 * G], F8)
    nc.sync.dma_start(out=W2a, in_=W2ad)
    W2b = const.tile([P, 2 * G], F8)
    nc.sync.dma_start(out=W2b, in_=W2bd)
    b1c = const.tile([P, 8], f32)
    nc.sync.dma_start(out=b1c, in_=b1cd)
    b2c = const.tile([P, 8], f32)
    nc.sync.dma_start(out=b2c, in_=b2cd)
    Wd = const.tile([P, 2 * C], DT)
    for k in range(2):
        nc.sync.dma_start(out=Wd[:, k * C:(k + 1) * C],
                          in_=Wdd[P * k: P * (k + 1), :])
    bdc = const.tile([C, 1], f32)
    nc.sync.dma_start(out=bdc, in_=bdd)
    id80 = const.tile([C, C], f32)
    nc.sync.dma_start(out=id80, in_=ident)
    iotalab = const.tile([P, C], f32)
    nc.sync.dma_start(out=iotalab, in_=iotalabd)
    labT = const.tile([P, B // P], f32)
    nc.sync.dma_start(out=labT, in_=labelsT)

    # ---------------- pools for the recurrent loop ----------------
    states = tc.alloc_tile_pool(name="states", bufs=2)
    gates = tc.alloc_tile_pool(name="gates", bufs=2)
    pgate = tc.alloc_tile_pool(name="pgate", bufs=4, space="PSUM")
    feats = tc.alloc_tile_pool(name="feats", bufs=3)

    h1 = c1 = h2 = c2 = None

    def oh_build(t):
        # featrep already holds the one-hot (built host-side)
        oh = feats.tile([C, B], DT, tag="oh", name=f"oh_{t}")
        nc.sync.dma_start(out=oh, in_=featrep[t])
        return oh

    def l1_block(t, oh, h1_in):
        sg1 = []
        for g in range(4):
            sg = gates.tile([P, 2 * B], DT, tag=f"sg_{g}", name=f"sg1_{g}")
            for ml in range(2):
                m = 2 * g + ml
                ps = pgate.tile([P, B], f32, tag="g", name=f"ps1_{t}_{g}_{ml}")
                for n in range(NB):
                    nc.tensor.matmul(ps[:, 512 * n: 512 * (n + 1)],
                                     E1[:, P * m: P * (m + 1)],
                                     oh[:, 512 * n: 512 * (n + 1)],
                                     start=True, stop=(t == 0))
                if t > 0 and thin != "pe":
                    for n in range(NB):
                        nc.tensor.matmul(ps[:, 512 * n: 512 * (n + 1)],
                                         drw(W1h, m), drm(h1_in, n),
                                         start=False, stop=True, perf_mode=DRM)
                act(sg[:, ml * B:(ml + 1) * B], ps, GATE_FUNCS[g],
                    bias=b1c[:, m:m + 1], scale=INV)
            sg1.append(sg)
        return sg1

    def l2_block(t, h1_in, h2_in):
        sg2 = []
        for g in range(4):
            sg = gates.tile([P, 2 * B], DT, tag=f"sg_{g}", name=f"sg2_{g}")
            for ml in range(2):
                m = 2 * g + ml
                ps = pgate.tile([P, B], f32, tag="g", name=f"ps2_{t}_{g}_{ml}")
                for n in range(NB):
                    nc.tensor.matmul(ps[:, 512 * n: 512 * (n + 1)],
                                     drw(W2a, m), drm(h1_in, n),
                                     start=True, stop=(t == 0), perf_mode=DRM)
                if t > 0 and thin != "pe":
                    for n in range(NB):
                        nc.tensor.matmul(ps[:, 512 * n: 512 * (n + 1)],
                                         drw(W2b, m), drm(h2_in, n),
                                         start=False, stop=True, perf_mode=DRM)
                act(sg[:, ml * B:(ml + 1) * B], ps,
                    GATE_FUNCS[g], bias=b2c[:, m:m + 1], scale=INV)
            sg2.append(sg)
        return sg2

    def cell(t, sgates, c_in, ctag, htag, thname):
        # Chunked along batch so chunk 0's h is ready for the next step's
        # matmuls while chunk 1 is still flowing through DVE/ACT.
        si, sj, sf, so = sgates
        cn = states.tile([P, 2 * B], DT, tag=ctag, name=ctag)
        th = gates.tile([P, 2 * B], DT, tag="th", name=thname, bufs=2)
        hn = states.tile([P, 2 * B], F8, tag=htag, name=htag)
        for n in range(NB):
            if thin in ("dve", "act"):
                v = lambda t2d: t2d[:, 64 * n:64 * n + 32]
            else:
                v = lambda t2d: drm(t2d, n)
            nc.vector.tensor_tensor(v(si), v(si), v(sj), op=OP.mult)
            if t == 0:
                nc.vector.tensor_copy(v(cn), v(si))
            else:
                nc.vector.tensor_tensor(v(sf), v(c_in), v(sf), op=OP.mult)
                nc.vector.tensor_tensor(v(cn), v(sf), v(si), op=OP.add)
            nc.scalar.activation(v(th), v(cn), AF.Tanh)
            nc.vector.tensor_tensor(v(hn), v(th), v(so), op=OP.mult)
        return cn, hn, th, so

    # Software pipeline: L2 runs one step behind L1.  L1's cell is emitted
    # before L2's block so tanh(c1) is not queued behind L2's gate acts.
    oh = oh_build(0)
    sg2_pend = None
    for t in range(T_steps):
        h1_prev = h1
        sg1 = l1_block(t, oh, h1_prev)
        c1, h1, _, _ = cell(t, sg1, c1, "c1", "h1", "th1")
        if t + 1 < T_steps:
            oh = oh_build(t + 1)                 # prefetch, off the h1 chain
        if t > 0:
            sg2_pend = l2_block(t - 1, h1_prev, h2)
            c2, h2, _, _ = cell(t - 1, sg2_pend, c2, "c2", "h2", "th2")

    # drain the pipeline: L2 for the final step
    sg2_pend = l2_block(T_steps - 1, h1, h2)
    c2, h2, th2, so2 = cell(T_steps - 1, sg2_pend, c2, "c2", "h2", "th2")

    # recompute the final h2 in bf16 for the dense layer
    h2bf = gates.tile([P, 2 * B], DT, tag="h2bf", bufs=1)
    nc.vector.tensor_tensor(h2bf, th2, so2, op=OP.mult)

    feats.release()
    pgate.release()

    # ---------------- loss ----------------
    ploss = tc.alloc_tile_pool(name="ploss", bufs=1, space="PSUM")
    lpool = tc.alloc_tile_pool(name="lpool", bufs=2)

    lps = ploss.tile([C, B], f32, tag="logits")
    for n in range(NB):
        for k in range(2):
            nc.tensor.matmul(
                lps[:, 512 * n: 512 * (n + 1)],
                Wd[:, C * k: C * (k + 1)],
                h2bf[:, B * k + 512 * n: B * k + 512 * (n + 1)],
                start=(k == 0), stop=(k == 1))
    logits = lpool.tile([C, B], f32, tag="logits_sb", bufs=1)
    nc.scalar.activation(logits, lps, AF.Identity, bias=bdc[:, 0:1])

    loss_sb = lpool.tile([P, B // P], f32, tag="loss_sb", bufs=1)
    # per-chunk sum-exps / label logits gathered as COLUMNS of shared tiles so
    # the log and the final subtract are single ops (one Exp->Ln table switch)
    sumexp_all = lpool.tile([P, B // P], f32, tag="sumexp_all", bufs=1)
    lablog_all = lpool.tile([P, B // P], f32, tag="lablog_all", bufs=1)
    for cb in range(B // P):
        lt = ploss.tile([P, C], f32, tag="lt", bufs=2, name=f"lt_{cb}")
        nc.tensor.transpose(lt, logits[:, P * cb: P * (cb + 1)], id80)
        ohl = lpool.tile([P, C], f32, tag="ohl", name=f"ohl_{cb}")
        nc.vector.tensor_scalar(ohl, iotalab, labT[:, cb:cb + 1], None,
                                op0=OP.is_equal)
        scr1 = lpool.tile([P, C], f32, tag="scr1", name=f"scr1_{cb}")
        nc.vector.scalar_tensor_tensor(scr1, lt, 1.0, ohl,
                                       op0=OP.mult, op1=OP.mult,
                                       accum_out=lablog_all[:, cb:cb + 1])
        scr2 = lpool.tile([P, C], f32, tag="scr2", name=f"scr2_{cb}")
        nc.scalar.activation(scr2, lt, AF.Exp,
                             accum_out=sumexp_all[:, cb:cb + 1])
    lse = lpool.tile([P, B // P], f32, tag="lse", bufs=1)
    nc.scalar.activation(lse, sumexp_all, AF.Ln)
    nc.vector.tensor_sub(loss_sb, lse, lablog_all)
    nc.sync.dma_start(out=lossd, in_=loss_sb)
    lpool.release()
    ploss.release()
    gates.release()
    states.release()
    const.release()


# ---------------------------------------------------------------------------
# host side
# ---------------------------------------------------------------------------
_CACHE = {}


def _get_program():
    if "nc" not in _CACHE:
        _CACHE["nc"] = build_program()
    return _CACHE["nc"]


def make_in_maps(features, labels, embedding, W1, b1, W2, b2, Wd, bd):
    """Shard the full inputs into 8 per-core input maps."""
    import ml_dtypes
    f32 = np.float32
    fp8 = ml_dtypes.float8_e4m3
    bf16 = ml_dtypes.bfloat16
    features = np.asarray(features, dtype=np.int32)
    labels = np.asarray(labels, dtype=np.int32)
    emb32 = np.asarray(embedding, f32)
    W132 = np.asarray(W1, f32)
    W232 = np.asarray(W2, f32)

    def fold2(mat):  # [256, G] -> [128, 2G] with k-tile-major columns
        if SWI:
            # per gate-block m: [A127, B127, ..., A0, B0] (A/B = k-tiles,
            # columns reversed) — the hw DoubleRowSwInterleave weight layout
            wf = mat.reshape(2, P, 8, P)[..., ::-1]
            return np.ascontiguousarray(
                wf.transpose(1, 2, 3, 0).reshape(P, 2 * G))
        return np.ascontiguousarray(
            mat.reshape(2, P, G).transpose(1, 0, 2).reshape(P, 2 * G))

    E1bf = np.ascontiguousarray(WSCALE * (emb32 @ W132[:E])).astype(bf16)
    W1h8 = fold2(WSCALE * W132[E:]).astype(fp8)
    W28a = fold2(WSCALE * W232[:H]).astype(fp8)
    W28b = fold2(WSCALE * W232[H:]).astype(fp8)
    b1c = np.ascontiguousarray(np.asarray(b1, f32).reshape(8, P).T)
    b1c[:, 4:6] += 1.0                          # forget-gate bias
    b2c = np.ascontiguousarray(np.asarray(b2, f32).reshape(8, P).T)
    b2c[:, 4:6] += 1.0

    w8 = np.ascontiguousarray(np.concatenate([W1h8, W28a, W28b], axis=1))
    cbf = np.zeros((2 * P, G + C), bf16)
    cbf[0:C, 0:G] = E1bf
    cbf[:, G:G + C] = np.asarray(Wd, f32).astype(bf16)
    cf32 = np.zeros((P, 186), f32)
    cf32[:, 0:8] = b1c
    cf32[:, 8:16] = b2c
    cf32[:, 16:96] = np.arange(C, dtype=f32)[None, :]
    cf32[0:C, 104:184] = np.eye(C, dtype=f32)
    cf32[0:C, 184] = np.arange(C, dtype=f32)
    cf32[0:C, 185] = np.asarray(bd, f32)

    in_maps = []
    for c in range(N_CORES):
        fs = features[B * c: B * (c + 1)]            # [B, T]
        ls = labels[B * c: B * (c + 1)]              # [B]
        ft = fs.T                                    # [T, B]
        featrep = np.ascontiguousarray(
            (ft[:, None, :] == np.arange(C, dtype=np.int32)[None, :, None])
            .astype(bf16))
        cfc = cf32.copy()
        cfc[:, 96:96 + B // P] = ls.reshape(B // P, P).T.astype(f32)
        blob = np.zeros((2 * P, 4548), bf16)
        blob[0:P, 0:3 * G] = w8.view(bf16)
        blob[0:P, 3 * G:3 * G + 372] = cfc.view(bf16)
        blob[:, 3 * G + 372:3 * G + 372 + G + C] = cbf
        in_maps.append({"blob": blob, "featrep": featrep})
    return in_maps


def gather_output(results):
    outs = []
    for r in results:
        outs.append(np.asarray(r["loss"]).T.reshape(-1))   # [P, B//P] -> [B]
    return np.concatenate(outs, axis=0).astype(np.float32)


def kernel(features, labels, embedding, W1, b1, W2, b2, Wd, bd):
    nc = _get_program()
    in_maps = make_in_maps(features, labels, embedding, W1, b1, W2, b2, Wd, bd)
    res = run_bass_kernel_spmd(nc, in_maps, core_ids=list(range(N_CORES)))
    return gather_output(res.results)
